# revision 27
# baseline (speedup 1.0000x reference)
"""Weighted-BCE loss kernel for Trainium2 (8 NeuronCores, SPMD data-parallel).

Reference math (torch-style BCELoss with class-balancing weights):
    n   = len(x), s = sum(gt)
    w0  = n / (2*(n-s)),  w1 = n / (2*s)
    L1  = max(log(x),     -100)
    L0  = max(log1p(-x),  -100)
    loss = mean( where(gt==0, w0, w1) * -(gt*L1 + (1-gt)*L0) )

Only ONE of log(x) / log(1-x) is needed per element (selected by gt), so
instead of two Ln passes we compute the selected operand in one shot:
    z = gt ? x : 1-x  =  1 - |x' - gt|,   x' = max(x, 2^-24)
(the clamp rides the op0 slot of the w-STT for free and guarantees
z >= 2^-24, so Ln never sees 0; vs the reference's -100 clamp this only
misvalues exact x==0 elements - ~1 in 16.7M, error ~5e-6 of the loss).

Global sums, all computed shard-locally (weights need only GLOBAL s):
    A  = sum(gt * Lz)        [DVE STT accum]  = sum_{gt=1} log x
    T  = sum(Lz)             [ACT accum, free on the Ln pass]
    W  = sum(x' - gt)        [free accum on the w-STT]
    Sx = sum(x)              [PE column-sum matmuls -> one PSUM bank]
    s  = Sx - W  (exact to ~1e-6 rel), so no separate sum(gt) pass.
    loss = -( A/(2s) + (T-A)/(2(n-s)) )

Dataflow (measured: DVE STT 1.08ns/col + ~0.15us/op, ACT pass 0.98ns/col,
DMA 429 GB/s with 16KB descriptors, less for smaller):
  - x and gt live in two fully-RESIDENT SBUF tensors (64KB/partition
    each).  All DMAs are pre-issued on the single SP HWDGE queue as
    interleaved x/gt chunk pairs (ramping chunk sizes: small first pair
    so compute starts ~11us, 4096-col chunks mid-stream for peak BW).
    Nothing downstream can ever stall the queue - compute sub-tiles
    just wait on the covering chunk-completion semaphores.
  - per sub-tile: DVE w-STT writes w; ACT Abs (in place) then
    Ln(1 - d) (in place, accum T) turn it into Lz; DVE A-STT (deferred
    two sub-tiles to stay decoupled from the cross-engine chain)
    reads Lz + gt.  In-place activations mean one [128, tfd] working
    tile per sub-tile: SBUF = 128K resident + ~56K working.
  - PE accumulates [128,512] column-sum matmuls of x into one PSUM
    bank; a dummy Ln in the preamble pre-loads the natural_log act
    table (abs/ln/copy) so no table swap lands mid-pipeline.
Host gathers the [128, 3*NT] accums + [1, 512] PSUM colsum from all 8
cores and finishes the (tiny) all-reduce + scalar math in float64.
"""

import numpy as np
from contextlib import ExitStack

import concourse.bass as bass
import concourse.bacc as bacc
import concourse.mybir as mybir
import concourse.tile as tile
from concourse.alu_op_type import AluOpType
from concourse.bass_utils import run_bass_kernel_spmd

N_TOTAL = 16777216
N_CORES = 8
PER_CORE = N_TOTAL // N_CORES   # 2097152
P = 128
FD = PER_CORE // P              # 16384 free elements per partition
# DMA chunk schedule, issued as interleaved x/gt pairs on one queue
CHUNKS = [1024, 1024, 2048, 4096, 4096, 4096]
assert sum(CHUNKS) == FD
# compute sub-tiles; each must lie inside a single DMA chunk
TILE_SIZES = [1024, 1024, 2048, 2048, 2048, 2048, 2048, 2048, 1024, 512, 512]
assert sum(TILE_SIZES) == FD
NT = len(TILE_SIZES)
MM = 512                        # moving free-dim chunk for PE colsums
X_CLAMP = 5.9604645e-08         # 2^-24: keeps z = 1-|x'-gt| >= 2^-24
LOG_CLAMP = -100.0

# Optional instrumentation knobs for a driver script (harness never sets them).
TRACE = False
LAST_RESULTS = None

_NC_CACHE = None


def _build():
    f32 = mybir.dt.float32
    i32 = mybir.dt.int32
    Ln = mybir.ActivationFunctionType.Ln
    Abs = mybir.ActivationFunctionType.Abs

    nc = bacc.Bacc("TRN2")
    x_in = nc.declare_dram_parameter("x", [P, FD], f32, isOutput=False)
    g_in = nc.declare_dram_parameter("gt", [P, FD], i32, isOutput=False)
    # packed accum output: columns [A | T | W], NT each
    out_all = nc.declare_dram_parameter("out_all", [P, 3 * NT], f32, isOutput=True)
    sum_x = nc.declare_dram_parameter("sum_x", [1, MM], f32, isOutput=True)

    n_mm = FD // MM

    with tile.TileContext(nc) as tc, ExitStack() as ctx:
        resp = ctx.enter_context(tc.tile_pool(name="resp", bufs=1))
        wp = ctx.enter_context(tc.tile_pool(name="wp", bufs=6))
        jp = ctx.enter_context(tc.tile_pool(name="jp", bufs=2))
        accp = ctx.enter_context(tc.tile_pool(name="accp", bufs=1))
        pp = ctx.enter_context(tc.psum_pool(name="pp", bufs=1))

        # fully-resident input tensors
        x_sb = resp.tile([P, FD], f32)
        g_sb = resp.tile([P, FD], i32)

        # pre-issue every DMA on the single SP queue as x/gt pairs
        off = 0
        for cw in CHUNKS:
            cs, ce = off, off + cw
            off += cw
            nc.sync.dma_start(x_sb[:, cs:ce], x_in[:, cs:ce])
            nc.sync.dma_start(g_sb[:, cs:ce], g_in[:, cs:ce])

        # one packed accum tile -> one output DMA
        acc_all = accp.tile([P, 3 * NT], f32)

        ones = accp.tile([P, 1], f32)
        nc.gpsimd.memset(ones[:], 1.0)

        # dummy Ln: forces the natural_log act-table (contains abs/ln/copy)
        # to load during the preamble instead of mid-pipeline
        warm = accp.tile([P, 1], f32)
        nc.scalar.activation(warm[:], ones[:], Ln)

        psum_t = pp.tile([1, MM], f32)

        def col(group, i):
            return acc_all[:, group * NT + i : group * NT + i + 1]

        def emit_A(i, lz, gsl, tfd):
            junk_a = jp.tile([P, tfd], f32, tag="junk_a")
            nc.vector.scalar_tensor_tensor(
                junk_a[:], lz[:], LOG_CLAMP, g_sb[:, gsl],
                AluOpType.max, AluOpType.mult,
                accum_out=col(0, i),
            )

        pending_A = []  # (i, lz_tile, gt_slice, tfd): emitted 2 sub-tiles late
        mm_idx = 0
        off = 0
        for i, tfd in enumerate(TILE_SIZES):
            sl = slice(off, off + tfd)
            off += tfd

            # w = max(x, 2^-24) - gt in [-1, 1];  accum -> W
            wt = wp.tile([P, tfd], f32, tag="w")
            nc.vector.scalar_tensor_tensor(
                wt[:], x_sb[:, sl], X_CLAMP, g_sb[:, sl],
                AluOpType.max, AluOpType.subtract,
                accum_out=col(2, i),
            )
            # Sx: accumulate column sums of x into one PSUM bank (idle PE)
            for c in range(sl.start, sl.stop, MM):
                nc.tensor.matmul(
                    psum_t[:], ones[:], x_sb[:, c : c + MM],
                    start=(mm_idx == 0), stop=(mm_idx == n_mm - 1),
                )
                mm_idx += 1
            # in place on ACT: d = |w|, then Lz = Ln(1 - d), accum -> T
            nc.scalar.activation(wt[:], wt[:], Abs)
            nc.scalar.activation(
                wt[:], wt[:], Ln, bias=1.0, scale=-1.0,
                accum_out=col(1, i),
            )
            # A-STT deferred three sub-tiles: by the time it reaches the head
            # of DVE's in-order queue its Ln input is certainly done, so it
            # never head-blocks ready w-STTs behind it
            pending_A.append((i, wt, sl, tfd))
            if len(pending_A) > 3:
                emit_A(*pending_A.pop(0))

        # psum -> SBUF copy early (only needs the last matmul, not the As)
        sum_x_sb = accp.tile([1, MM], f32)
        nc.scalar.copy(sum_x_sb[:], psum_t[:])
        nc.sync.dma_start(sum_x[:, :], sum_x_sb[:])

        for args in pending_A:
            emit_A(*args)

        nc.sync.dma_start(out_all[:, :], acc_all[:])

    nc.compile()
    return nc


def get_nc():
    global _NC_CACHE
    if _NC_CACHE is None:
        _NC_CACHE = _build()
    return _NC_CACHE


def make_in_maps(x, gt):
    x = np.ascontiguousarray(np.asarray(x, dtype=np.float32).reshape(-1))
    gt = np.ascontiguousarray(np.asarray(gt, dtype=np.int32).reshape(-1))
    assert x.shape == (N_TOTAL,) and gt.shape == (N_TOTAL,)
    in_maps = []
    for c in range(N_CORES):
        sl = slice(c * PER_CORE, (c + 1) * PER_CORE)
        in_maps.append({
            "x": x[sl].reshape(P, FD),
            "gt": gt[sl].reshape(P, FD),
        })
    return in_maps


def combine(results):
    """All-reduce the per-core partial sums and finish the loss formula."""
    A = T = S = 0.0
    for r in results:
        o = r["out_all"].astype(np.float64)
        A += o[:, 0 * NT : 1 * NT].sum()
        T += o[:, 1 * NT : 2 * NT].sum()
        W = o[:, 2 * NT : 3 * NT].sum()
        Sx = r["sum_x"].astype(np.float64).sum()
        S += Sx - W                      # sum(gt) for this core
    n = float(N_TOTAL)
    result = -(A / (2.0 * S) + (T - A) / (2.0 * (n - S)))
    return np.array(result, dtype=np.float32)


def kernel(x, gt):
    global LAST_RESULTS
    nc = get_nc()
    in_maps = make_in_maps(x, gt)
    br = run_bass_kernel_spmd(nc, in_maps, list(range(N_CORES)))
    LAST_RESULTS = br
    return combine(br.results)


# revision 30
# speedup vs baseline: 1.1097x; 1.1097x over previous
"""Weighted-BCE loss kernel for Trainium2 (8 NeuronCores, SPMD data-parallel).

Reference math (torch-style BCELoss with class-balancing weights):
    n   = len(x), s = sum(gt)
    w0  = n / (2*(n-s)),  w1 = n / (2*s)
    L1  = max(log(x),     -100)
    L0  = max(log1p(-x),  -100)
    loss = mean( where(gt==0, w0, w1) * -(gt*L1 + (1-gt)*L0) )

Only ONE of log(x) / log(1-x) is needed per element (selected by gt), so
instead of two Ln passes we compute the selected operand in one shot:
    z = gt ? x : 1-x  =  1 - |x - gt|
The Ln pass uses bias 1 + 2^-24, so even the x==0, gt==1 corner (where
|w| == 1 exactly) stays finite: Ln sees 2^-24 -> -16.6.  vs the
reference's -100 clamp this misvalues only exact x==0 elements (~1 in
16.7M, error ~5e-6 of the loss); the +2^-24 shift itself biases the
mean by ~1e-6.  Global sums, all computed shard-locally:
    A  = sum(gt * Lz)   [DVE STT accum]   = sum_{gt=1} log x
    T  = sum(Lz)        [ACT accum, free on the Ln pass]
    s  = sum(x) - sum(w)  [PE colsum matmuls -> 2 PSUM banks, one DVE
                           psum-subtract at the end;  w = x - gt]
    loss = -( A/(2s) + (T-A)/(2(n-s)) )

Engine split - every elementwise pass on a DIFFERENT engine so each
engine runs ONE pass per element (measured: DVE STT 1.08ns/col, ACT
0.98ns/col, DMA 429 GB/s with 16KB descriptors only):
    GpSimd  w = tensor_tensor(x, gt, subtract)       (ucode, ~0.5 eff)
    ACT     d = Abs(w) -> its own tile; Lz = Ln(1+2^-24 - d) in place,
            accum -> T
    DVE     A-STT (Lz, gt) accum -> A, deferred 2 sub-tiles
    PE      per 512 cols: colsum(x) -> bank_x, colsum(w) -> bank_w,
            same `ones` stationary (no stationary reloads)
    SP      ALL input DMAs, pre-issued upfront as interleaved x/gt
            chunk pairs into two fully-RESIDENT SBUF tensors (64KB/
            partition each) - nothing downstream can stall the queue;
            ramping chunk sizes so compute starts ~11us.
A dummy Ln in the preamble pre-loads the natural_log act table
(abs/ln/copy) so no table swap lands mid-pipeline.  Host gathers the
[128, 2*NT] accums + the [1, 512] S-colsums from all 8 cores and
finishes the (tiny) all-reduce + scalar math in float64.
"""

import numpy as np
from contextlib import ExitStack

import concourse.bass as bass
import concourse.bacc as bacc
import concourse.mybir as mybir
import concourse.tile as tile
from concourse.alu_op_type import AluOpType
from concourse.bass_utils import run_bass_kernel_spmd

N_TOTAL = 16777216
N_CORES = 8
PER_CORE = N_TOTAL // N_CORES   # 2097152
P = 128
FD = PER_CORE // P              # 16384 free elements per partition
# DMA chunk schedule, issued as interleaved x/gt pairs on one queue
CHUNKS = [1024, 1024, 2048, 4096, 4096, 4096]
assert sum(CHUNKS) == FD
# compute sub-tiles; each must lie inside a single DMA chunk
TILE_SIZES = [1024, 1024, 2048, 2048, 2048, 2048, 2048, 2048, 1024, 512, 512]
assert sum(TILE_SIZES) == FD
NT = len(TILE_SIZES)
MM = 512                        # moving free-dim chunk for PE colsums
LN_BIAS = 1.0 + 2.0**-23        # keeps Ln input >= 2^-23 even at |w| == 1
                                # (1 + 2^-24 would round to 1.0 in f32!)
LOG_CLAMP = -100.0
W_ON_GPSIMD = True              # fallback knob: False -> w-STT on DVE

# Optional instrumentation knobs for a driver script (harness never sets them).
TRACE = False
LAST_RESULTS = None

_NC_CACHE = None


def _build():
    f32 = mybir.dt.float32
    i32 = mybir.dt.int32
    Ln = mybir.ActivationFunctionType.Ln
    Abs = mybir.ActivationFunctionType.Abs

    nc = bacc.Bacc("TRN2")
    x_in = nc.declare_dram_parameter("x", [P, FD], f32, isOutput=False)
    g_in = nc.declare_dram_parameter("gt", [P, FD], i32, isOutput=False)
    # packed accum output: columns [A | T], NT each
    out_all = nc.declare_dram_parameter("out_all", [P, 2 * NT], f32, isOutput=True)
    # column sums of (x - w) = gt, summed on host -> s
    sum_s = nc.declare_dram_parameter("sum_s", [1, MM], f32, isOutput=True)

    n_mm = FD // MM

    with tile.TileContext(nc) as tc, ExitStack() as ctx:
        resp = ctx.enter_context(tc.tile_pool(name="resp", bufs=1))
        wp = ctx.enter_context(tc.tile_pool(name="wp", bufs=3))
        dp = ctx.enter_context(tc.tile_pool(name="dp", bufs=4))
        jp = ctx.enter_context(tc.tile_pool(name="jp", bufs=2))
        accp = ctx.enter_context(tc.tile_pool(name="accp", bufs=1))
        pp = ctx.enter_context(tc.psum_pool(name="pp", bufs=1))

        # fully-resident input tensors
        x_sb = resp.tile([P, FD], f32)
        g_sb = resp.tile([P, FD], i32)

        # pre-issue every DMA on the single SP queue as x/gt pairs
        off = 0
        for cw in CHUNKS:
            cs, ce = off, off + cw
            off += cw
            nc.sync.dma_start(x_sb[:, cs:ce], x_in[:, cs:ce])
            nc.sync.dma_start(g_sb[:, cs:ce], g_in[:, cs:ce])

        # one packed accum tile -> one output DMA
        acc_all = accp.tile([P, 2 * NT], f32)

        ones = accp.tile([P, 1], f32)
        nc.gpsimd.memset(ones[:], 1.0)
        ln_bias = accp.tile([P, 1], f32)
        nc.vector.memset(ln_bias[:], LN_BIAS)

        # dummy Ln: forces the natural_log act-table (contains abs/ln/copy)
        # to load during the preamble instead of mid-pipeline
        warm = accp.tile([P, 1], f32)
        nc.scalar.activation(warm[:], ones[:], Ln)

        bank_x = pp.tile([1, MM], f32)
        bank_w = pp.tile([1, MM], f32)

        def col(group, i):
            return acc_all[:, group * NT + i : group * NT + i + 1]

        def emit_A(i, lz, gsl, tfd):
            junk_a = jp.tile([P, tfd], f32, tag="junk_a")
            nc.vector.scalar_tensor_tensor(
                junk_a[:], lz[:], LOG_CLAMP, g_sb[:, gsl],
                AluOpType.max, AluOpType.mult,
                accum_out=col(0, i),
            )

        pending_A = []  # (i, lz_tile, gt_slice, tfd): emitted 2 sub-tiles late
        mmx = 0
        mmw = 0
        off = 0
        for i, tfd in enumerate(TILE_SIZES):
            sl = slice(off, off + tfd)
            off += tfd

            # w = x - gt in (-1, 1]  (no clamp needed: Ln bias covers |w|=1)
            wt = wp.tile([P, tfd], f32, tag="w")
            if W_ON_GPSIMD:
                nc.gpsimd.tensor_tensor(
                    wt[:], x_sb[:, sl], g_sb[:, sl], AluOpType.subtract
                )
            else:
                nc.vector.scalar_tensor_tensor(
                    wt[:], x_sb[:, sl], 0.0, g_sb[:, sl],
                    AluOpType.max, AluOpType.subtract,
                )
            # colsum(x) and colsum(w) into separate PSUM banks (same
            # stationary -> no reloads); host computes s = sum(x - w)
            for c in range(sl.start, sl.stop, MM):
                nc.tensor.matmul(
                    bank_x[:], ones[:], x_sb[:, c : c + MM],
                    start=(mmx == 0), stop=(mmx == n_mm - 1),
                )
                mmx += 1
            for c in range(0, tfd, MM):
                nc.tensor.matmul(
                    bank_w[:], ones[:], wt[:, c : c + MM],
                    start=(mmw == 0), stop=(mmw == n_mm - 1),
                )
                mmw += 1
            # ACT: d = |w|, then Lz = Ln(1 + 2^-24 - d) in place, accum -> T
            dt_ = dp.tile([P, tfd], f32, tag="d")
            nc.scalar.activation(dt_[:], wt[:], Abs)
            nc.scalar.activation(
                dt_[:], dt_[:], Ln, bias=ln_bias[:], scale=-1.0,
                accum_out=col(1, i),
            )
            # A-STT deferred two sub-tiles (DVE's only per-element work)
            pending_A.append((i, dt_, sl, tfd))
            if len(pending_A) > 2:
                emit_A(*pending_A.pop(0))

        # S colsums: copy bank_x to SBUF, then subtract bank_w (only one
        # PSUM operand is allowed per instruction), one DMA
        sx_sb = accp.tile([1, MM], f32)
        nc.scalar.copy(sx_sb[:], bank_x[:])
        sum_s_sb = accp.tile([1, MM], f32)
        nc.vector.tensor_tensor(
            sum_s_sb[:], sx_sb[:], bank_w[:], AluOpType.subtract
        )
        nc.sync.dma_start(sum_s[:, :], sum_s_sb[:])

        for args in pending_A:
            emit_A(*args)

        nc.sync.dma_start(out_all[:, :], acc_all[:])

    nc.compile()
    return nc


def get_nc():
    global _NC_CACHE
    if _NC_CACHE is None:
        _NC_CACHE = _build()
    return _NC_CACHE


def make_in_maps(x, gt):
    x = np.ascontiguousarray(np.asarray(x, dtype=np.float32).reshape(-1))
    gt = np.ascontiguousarray(np.asarray(gt, dtype=np.int32).reshape(-1))
    assert x.shape == (N_TOTAL,) and gt.shape == (N_TOTAL,)
    in_maps = []
    for c in range(N_CORES):
        sl = slice(c * PER_CORE, (c + 1) * PER_CORE)
        in_maps.append({
            "x": x[sl].reshape(P, FD),
            "gt": gt[sl].reshape(P, FD),
        })
    return in_maps


def combine(results):
    """All-reduce the per-core partial sums and finish the loss formula."""
    A = T = S = 0.0
    for r in results:
        o = r["out_all"].astype(np.float64)
        A += o[:, 0 * NT : 1 * NT].sum()
        T += o[:, 1 * NT : 2 * NT].sum()
        S += r["sum_s"].astype(np.float64).sum()
    n = float(N_TOTAL)
    result = -(A / (2.0 * S) + (T - A) / (2.0 * (n - S)))
    return np.array(result, dtype=np.float32)


def kernel(x, gt):
    global LAST_RESULTS
    nc = get_nc()
    in_maps = make_in_maps(x, gt)
    br = run_bass_kernel_spmd(nc, in_maps, list(range(N_CORES)))
    LAST_RESULTS = br
    return combine(br.results)


# revision 31
# speedup vs baseline: 1.1761x; 1.0599x over previous
"""Weighted-BCE loss kernel for Trainium2 (8 NeuronCores, SPMD data-parallel).

Reference math (torch-style BCELoss with class-balancing weights):
    n   = len(x), s = sum(gt)
    w0  = n / (2*(n-s)),  w1 = n / (2*s)
    L1  = max(log(x),     -100)
    L0  = max(log1p(-x),  -100)
    loss = mean( where(gt==0, w0, w1) * -(gt*L1 + (1-gt)*L0) )

Only ONE of log(x) / log(1-x) is needed per element (selected by gt), so
instead of two Ln passes we compute the selected operand in one shot:
    z = gt ? x : 1-x  =  1 - |x - gt|
The Ln pass uses bias 1 + 2^-24, so even the x==0, gt==1 corner (where
|w| == 1 exactly) stays finite: Ln sees 2^-24 -> -16.6.  vs the
reference's -100 clamp this misvalues only exact x==0 elements (~1 in
16.7M, error ~5e-6 of the loss); the +2^-24 shift itself biases the
mean by ~1e-6.  Global sums, all computed shard-locally:
    A  = sum(gt * Lz)   [DVE STT accum]   = sum_{gt=1} log x
    T  = sum(Lz)        [ACT accum, free on the Ln pass]
    s  = sum(x) - sum(w)  [PE colsum matmuls -> 2 PSUM banks, one DVE
                           psum-subtract at the end;  w = x - gt]
    loss = -( A/(2s) + (T-A)/(2(n-s)) )

Engine split - every elementwise pass on a DIFFERENT engine so each
engine runs ONE pass per element (measured: DVE STT 1.08ns/col, ACT
0.98ns/col, DMA 429 GB/s with 16KB descriptors only):
    GpSimd  w = tensor_tensor(x, gt, subtract)       (ucode, ~0.5 eff)
    ACT     d = Abs(w) -> its own tile; Lz = Ln(1+2^-24 - d) in place,
            accum -> T
    DVE     A-STT (Lz, gt) accum -> A, deferred 2 sub-tiles
    PE      per 512 cols: colsum(x) -> bank_x, colsum(w) -> bank_w,
            same `ones` stationary (no stationary reloads)
    SP      ALL input DMAs, pre-issued upfront as interleaved x/gt
            chunk pairs into two fully-RESIDENT SBUF tensors (64KB/
            partition each) - nothing downstream can stall the queue;
            ramping chunk sizes so compute starts ~11us.
A dummy Ln in the preamble pre-loads the natural_log act table
(abs/ln/copy) so no table swap lands mid-pipeline.  Host gathers the
[128, 2*NT] accums + the [1, 512] S-colsums from all 8 cores and
finishes the (tiny) all-reduce + scalar math in float64.
"""

import numpy as np
from contextlib import ExitStack

import concourse.bass as bass
import concourse.bacc as bacc
import concourse.mybir as mybir
import concourse.tile as tile
from concourse.alu_op_type import AluOpType
from concourse.bass_utils import run_bass_kernel_spmd

N_TOTAL = 16777216
N_CORES = 8
PER_CORE = N_TOTAL // N_CORES   # 2097152
P = 128
FD = PER_CORE // P              # 16384 free elements per partition
# DMA chunk schedule, issued as interleaved x/gt pairs on one queue
CHUNKS = [1024, 1024, 2048, 4096, 4096, 4096]
assert sum(CHUNKS) == FD
# compute sub-tiles; each must lie inside a single DMA chunk
TILE_SIZES = [1024, 1024, 2048, 2048, 2048, 2048, 2048, 2048, 1024, 512, 512]
assert sum(TILE_SIZES) == FD
NT = len(TILE_SIZES)
MM = 512                        # moving free-dim chunk for PE colsums
LN_BIAS = 1.0 + 2.0**-23        # keeps Ln input >= 2^-23 even at |w| == 1
                                # (1 + 2^-24 would round to 1.0 in f32!)
LOG_CLAMP = -100.0
W_ON_GPSIMD = False             # gpsimd tensor_tensor measured ~0.3 eff
                                # AND its SBUF traffic slows DVE/ACT; the
                                # DVE STT is strictly better

# Optional instrumentation knobs for a driver script (harness never sets them).
TRACE = False
LAST_RESULTS = None

_NC_CACHE = None


def _build():
    f32 = mybir.dt.float32
    i32 = mybir.dt.int32
    Ln = mybir.ActivationFunctionType.Ln
    Abs = mybir.ActivationFunctionType.Abs

    nc = bacc.Bacc("TRN2")
    x_in = nc.declare_dram_parameter("x", [P, FD], f32, isOutput=False)
    g_in = nc.declare_dram_parameter("gt", [P, FD], i32, isOutput=False)
    # packed accum output: columns [A | T], NT each
    out_all = nc.declare_dram_parameter("out_all", [P, 2 * NT], f32, isOutput=True)
    # column sums of (x - w) = gt, summed on host -> s
    sum_s = nc.declare_dram_parameter("sum_s", [1, MM], f32, isOutput=True)

    n_mm = FD // MM

    with tile.TileContext(nc) as tc, ExitStack() as ctx:
        resp = ctx.enter_context(tc.tile_pool(name="resp", bufs=1))
        wp = ctx.enter_context(tc.tile_pool(name="wp", bufs=3))
        dp = ctx.enter_context(tc.tile_pool(name="dp", bufs=4))
        jp = ctx.enter_context(tc.tile_pool(name="jp", bufs=2))
        accp = ctx.enter_context(tc.tile_pool(name="accp", bufs=1))
        pp = ctx.enter_context(tc.psum_pool(name="pp", bufs=1))

        # fully-resident input tensors
        x_sb = resp.tile([P, FD], f32)
        g_sb = resp.tile([P, FD], i32)

        # pre-issue every DMA on the single SP queue as x/gt pairs
        off = 0
        for cw in CHUNKS:
            cs, ce = off, off + cw
            off += cw
            nc.sync.dma_start(x_sb[:, cs:ce], x_in[:, cs:ce])
            nc.sync.dma_start(g_sb[:, cs:ce], g_in[:, cs:ce])

        # one packed accum tile -> one output DMA
        acc_all = accp.tile([P, 2 * NT], f32)

        ones = accp.tile([P, 1], f32)
        nc.gpsimd.memset(ones[:], 1.0)
        ln_bias = accp.tile([P, 1], f32)
        nc.vector.memset(ln_bias[:], LN_BIAS)

        # dummy Ln: forces the natural_log act-table (contains abs/ln/copy)
        # to load during the preamble instead of mid-pipeline
        warm = accp.tile([P, 1], f32)
        nc.scalar.activation(warm[:], ones[:], Ln)

        bank_x = pp.tile([1, MM], f32)
        bank_w = pp.tile([1, MM], f32)

        def col(group, i):
            return acc_all[:, group * NT + i : group * NT + i + 1]

        def emit_A(i, lz, gsl, tfd):
            junk_a = jp.tile([P, tfd], f32, tag="junk_a")
            nc.vector.scalar_tensor_tensor(
                junk_a[:], lz[:], LOG_CLAMP, g_sb[:, gsl],
                AluOpType.max, AluOpType.mult,
                accum_out=col(0, i),
            )

        pending_A = []  # (i, lz_tile, gt_slice, tfd): emitted 2 sub-tiles late
        mmx = 0
        mmw = 0
        off = 0
        for i, tfd in enumerate(TILE_SIZES):
            sl = slice(off, off + tfd)
            off += tfd

            # w = x - gt in (-1, 1]  (no clamp needed: Ln bias covers |w|=1)
            wt = wp.tile([P, tfd], f32, tag="w")
            if W_ON_GPSIMD:
                nc.gpsimd.tensor_tensor(
                    wt[:], x_sb[:, sl], g_sb[:, sl], AluOpType.subtract
                )
            else:
                nc.vector.scalar_tensor_tensor(
                    wt[:], x_sb[:, sl], 0.0, g_sb[:, sl],
                    AluOpType.max, AluOpType.subtract,
                )
            # colsum(x) and colsum(w) into separate PSUM banks (same
            # stationary -> no reloads); host computes s = sum(x - w)
            for c in range(sl.start, sl.stop, MM):
                nc.tensor.matmul(
                    bank_x[:], ones[:], x_sb[:, c : c + MM],
                    start=(mmx == 0), stop=(mmx == n_mm - 1),
                )
                mmx += 1
            for c in range(0, tfd, MM):
                nc.tensor.matmul(
                    bank_w[:], ones[:], wt[:, c : c + MM],
                    start=(mmw == 0), stop=(mmw == n_mm - 1),
                )
                mmw += 1
            # ACT: d = |w|, then Lz = Ln(1 + 2^-24 - d) in place, accum -> T
            dt_ = dp.tile([P, tfd], f32, tag="d")
            nc.scalar.activation(dt_[:], wt[:], Abs)
            nc.scalar.activation(
                dt_[:], dt_[:], Ln, bias=ln_bias[:], scale=-1.0,
                accum_out=col(1, i),
            )
            # A-STT deferred two sub-tiles (DVE's only per-element work)
            pending_A.append((i, dt_, sl, tfd))
            if len(pending_A) > 2:
                emit_A(*pending_A.pop(0))

        # S colsums: copy bank_x to SBUF, then subtract bank_w (only one
        # PSUM operand is allowed per instruction), one DMA
        sx_sb = accp.tile([1, MM], f32)
        nc.scalar.copy(sx_sb[:], bank_x[:])
        sum_s_sb = accp.tile([1, MM], f32)
        nc.vector.tensor_tensor(
            sum_s_sb[:], sx_sb[:], bank_w[:], AluOpType.subtract
        )
        nc.sync.dma_start(sum_s[:, :], sum_s_sb[:])

        for args in pending_A:
            emit_A(*args)

        nc.sync.dma_start(out_all[:, :], acc_all[:])

    nc.compile()
    return nc


def get_nc():
    global _NC_CACHE
    if _NC_CACHE is None:
        _NC_CACHE = _build()
    return _NC_CACHE


def make_in_maps(x, gt):
    x = np.ascontiguousarray(np.asarray(x, dtype=np.float32).reshape(-1))
    gt = np.ascontiguousarray(np.asarray(gt, dtype=np.int32).reshape(-1))
    assert x.shape == (N_TOTAL,) and gt.shape == (N_TOTAL,)
    in_maps = []
    for c in range(N_CORES):
        sl = slice(c * PER_CORE, (c + 1) * PER_CORE)
        in_maps.append({
            "x": x[sl].reshape(P, FD),
            "gt": gt[sl].reshape(P, FD),
        })
    return in_maps


def combine(results):
    """All-reduce the per-core partial sums and finish the loss formula."""
    A = T = S = 0.0
    for r in results:
        o = r["out_all"].astype(np.float64)
        A += o[:, 0 * NT : 1 * NT].sum()
        T += o[:, 1 * NT : 2 * NT].sum()
        S += r["sum_s"].astype(np.float64).sum()
    n = float(N_TOTAL)
    result = -(A / (2.0 * S) + (T - A) / (2.0 * (n - S)))
    return np.array(result, dtype=np.float32)


def kernel(x, gt):
    global LAST_RESULTS
    nc = get_nc()
    in_maps = make_in_maps(x, gt)
    br = run_bass_kernel_spmd(nc, in_maps, list(range(N_CORES)))
    LAST_RESULTS = br
    return combine(br.results)
